# revision 16
# baseline (speedup 1.0000x reference)
"""Trainium2 Bass kernel for a GQA sliding-window attention layer.

Reference computation (B=2, T=2048, C=2048, 16 Q heads / 4 KV heads, d=128):
    q = x @ Wq; k = x @ Wk; v = x @ Wv (+ sigmoid-gated value embedding)
    q, k = rmsnorm(rope(q)), rmsnorm(rope(k))
    scores masked to the band 0 <= j - i < window (=1024), softmax over j
    out = (p @ v) @ Wo

Sharding: 8 cores = 2 batches x 4 KV groups.  Each core computes its 4 Q
heads / 1 KV head for one batch and a partial output (its 512-row slice of
the Wo contraction); the host sums the 4 partials per batch.

Layout strategy per core:
  - xT (C x T, bf16) resident in SBUF; all projections contract over C.
  - q-hat / k-hat kept [d=128 partitions, T free]; scores computed transposed
    (S^T tiles [kj, qi]) so that P^T feeds the PV matmul directly with v in
    natural [token, d] layout.
  - softmax has no max-subtraction: rms-normalized q,k bound |score| by
    sqrt(128); 1/sqrt(d) is folded into the exp's scale operand.
  - softmax denominators: exp tiles are accumulated elementwise (DVE), then
    one all-ones [128,128] f32r matmul per q-tile performs the key-reduction
    AND broadcasts the result across all partitions in one cheap matmul.
    The rms row-sums use the same all-ones trick (no [1,N] psum outputs, no
    separate broadcast matmuls).
  - band masking via 0/1 bf16 multiplies on GpSimd (no PE bias matmuls).
  - projections emitted as kt-major waves (first wave 8 groups wide so the
    PE chases the initial x/weight DMA stream at full rate).  Each wave's
    tail work is split: psum-releasing ops (rope mults) are emitted before
    the next wave's matmuls, tail PE ops are injected mid-wave, and the
    remainder trails the wave, so no engine queue ever deadlocks on a
    recycled psum bank.
"""

import numpy as np
import ml_dtypes
from collections import deque

BF16 = ml_dtypes.bfloat16

# Problem dims (hardcoded per contest rules)
B, T, C = 2, 2048, 2048
N_HEAD, N_KV, HD, GATE_CH = 16, 4, 128, 32
WINDOW = 1024
P = 128
GH = N_HEAD // N_KV  # q heads per kv head (= per core)
N_CORES = 8

_PROGRAM_CACHE = {}


def build_program(T_=T, C_=C, win=WINDOW):
    import concourse.mybir as mybir
    import concourse.tile as tile
    from concourse import bacc

    dt = mybir.dt
    f32 = dt.float32
    bf16 = dt.bfloat16
    AF = mybir.ActivationFunctionType
    ALU = mybir.AluOpType

    NT = T_ // P          # token tiles
    KT = C_ // P          # contraction tiles
    WT = win // P         # window tiles
    TS = T_ // 512        # 512-wide token slices
    ISQ = 1.0 / float(np.sqrt(HD))
    INJ = 8               # kt index where prev wave's tail PE ops interleave

    nc = bacc.Bacc()

    xT = nc.declare_dram_parameter("xT", [C_, T_], bf16, isOutput=False)
    wq = nc.declare_dram_parameter("wq", [C_, GH * HD], bf16, isOutput=False)
    wk = nc.declare_dram_parameter("wk", [C_, HD], bf16, isOutput=False)
    wv = nc.declare_dram_parameter("wv", [C_, HD], bf16, isOutput=False)
    wg = nc.declare_dram_parameter("wg", [GATE_CH, 1], bf16, isOutput=False)
    ve2 = nc.declare_dram_parameter("ve2", [T_, HD], bf16, isOutput=False)
    wo = nc.declare_dram_parameter("wo", [GH * HD, C_], bf16, isOutput=False)
    ccd = nc.declare_dram_parameter("cc", [P, T_], bf16, isOutput=False)
    ssd = nc.declare_dram_parameter("ss", [P, T_], bf16, isOutput=False)
    mlod = nc.declare_dram_parameter("mlo", [P, GH * P], bf16, isOutput=False)
    mhid = nc.declare_dram_parameter("mhi", [P, GH * P], bf16, isOutput=False)
    onebd = nc.declare_dram_parameter("onesb", [P, P], bf16, isOutput=False)
    idr = nc.declare_dram_parameter("identr", [P, P], bf16, isOutput=False)
    out_d = nc.declare_dram_parameter("out", [T_, C_], bf16, isOutput=True)

    with tile.TileContext(nc) as tc:
        with (
            tc.tile_pool(name="singles", bufs=1) as sg,
            tc.tile_pool(name="work", bufs=2) as wk_pool,
            tc.tile_pool(name="work3", bufs=3) as w3_pool,
            tc.tile_pool(name="attw", bufs=4) as aw,
            tc.tile_pool(name="outp", bufs=3) as op_pool,
            tc.tile_pool(name="psum", bufs=6, space="PSUM") as pp,
        ):
            # ---- persistent inputs -------------------------------------
            # small constants FIRST (rope tails read cc/ss early; masks and
            # the all-ones tiles are needed at the start of attention)
            wg_sb = sg.tile([GATE_CH, 1], bf16, tag="wg")
            nc.sync.dma_start(out=wg_sb[:], in_=wg[:])
            cc_sb = sg.tile([P, T_], bf16, tag="cc")
            nc.sync.dma_start(out=cc_sb[:], in_=ccd[:])
            ss_sb = sg.tile([P, T_], bf16, tag="ss")
            nc.sync.dma_start(out=ss_sb[:], in_=ssd[:])
            mlo_sb = sg.tile([P, GH * P], bf16, tag="mlo")
            nc.sync.dma_start(out=mlo_sb[:], in_=mlod[:])
            mhi_sb = sg.tile([P, GH * P], bf16, tag="mhi")
            nc.sync.dma_start(out=mhi_sb[:], in_=mhid[:])
            onesb_sb = sg.tile([P, P], bf16, tag="onesbb")
            nc.sync.dma_start(out=onesb_sb[:], in_=onebd[:])
            idr_sb = sg.tile([P, P], bf16, tag="idr")
            nc.sync.dma_start(out=idr_sb[:], in_=idr[:])
            xt = []
            wq_sb = sg.tile([P, KT, GH * HD], bf16, tag="wq")
            wk_sb = sg.tile([P, KT, HD], bf16, tag="wk")
            wv_sb = sg.tile([P, KT, HD], bf16, tag="wv")
            wqr = wq.rearrange("(o p) n -> p o n", p=P)
            wkr = wk.rearrange("(o p) n -> p o n", p=P)
            wvr = wv.rearrange("(o p) n -> p o n", p=P)
            for kt in range(KT):
                t_ = sg.tile([P, T_], bf16, tag=f"xt{kt}")
                nc.sync.dma_start(out=wk_sb[:, kt, :], in_=wkr[:, kt, :])
                nc.sync.dma_start(out=wv_sb[:, kt, :], in_=wvr[:, kt, :])
                for c4 in range(4):
                    cs = slice(c4 * 512, (c4 + 1) * 512)
                    nc.sync.dma_start(out=t_[:, cs],
                                      in_=xT[kt * P:(kt + 1) * P, cs])
                xt.append(t_)
            for kt in range(KT):
                nc.sync.dma_start(out=wq_sb[:, kt, :], in_=wqr[:, kt, :])
            ve2_sb = sg.tile([P, NT, HD], bf16, tag="ve2")
            nc.sync.dma_start(out=ve2_sb[:], in_=ve2.rearrange("(o p) d -> p o d", p=P))
            wo_sb = sg.tile([P, GH, C_], bf16, tag="wo")
            nc.sync.dma_start(out=wo_sb[:], in_=wo.rearrange("(o p) n -> p o n", p=P))
            eps_sb = sg.tile([P, 1], f32, tag="epsb")
            nc.vector.memset(eps_sb[:], 1e-6)

            # persistent intermediates
            qhat = sg.tile([P, GH, T_], bf16, tag="qhat")   # normalized roped q, [d, h, t]
            khat = sg.tile([P, T_], bf16, tag="khat")       # normalized roped k
            vsb = sg.tile([P, NT, HD], bf16, tag="vsb")     # gated v, [tok, tt, d]

            # ---- projections + rope + rmsnorm for k/q heads and vT -----
            # heads: 0 = k, 1..GH = q heads, GH+1 = v
            def wave_mms(wave, tags, inject):
                items = []
                for (head, ts_), tg in zip(wave, tags):
                    sl = slice(ts_ * 512, ts_ * 512 + 512)
                    ps = pp.tile([P, 512], f32, tag=tg, bufs=TAG_BUFS[tg],
                                 name=f"ps{head}_{ts_}")
                    items.append((head, sl, ps))
                for kt in range(KT):
                    if kt == INJ and inject is not None:
                        inject()
                    for gi, (head, ts_) in enumerate(wave):
                        if head == 0:
                            w_ap = wk_sb[:, kt, :]
                        elif head == GH + 1:
                            w_ap = wv_sb[:, kt, :]
                        else:
                            w_ap = wq_sb[:, kt, (head - 1) * HD:head * HD]
                        nc.tensor.matmul(
                            items[gi][2][:], lhsT=w_ap,
                            rhs=xt[kt][:, items[gi][1]],
                            start=(kt == 0), stop=(kt == KT - 1),
                        )
                return items

            def tails_free(items):
                """psum-releasing rope/copy work; emit BEFORE next wave's
                matmuls so the banks recycle while the next wave runs."""
                st = {"ropes": [], "vs": []}
                for (head, sl, ps) in items:
                    if head == GH + 1:
                        continue
                    # rope: qr = ps*cc + swap(ps)*ss (ss carries the sign);
                    # all on DVE (GpSimd has no PSUM port)
                    qr = w3_pool.tile([P, 512], f32, tag="qr", bufs=8)
                    nc.vector.tensor_mul(qr[:], ps[:], cc_sb[:, sl])
                    qs = wk_pool.tile([P, 512], f32, tag="qs", bufs=2)
                    nc.vector.tensor_mul(qs[0:64, :], ps[64:128, :],
                                         ss_sb[0:64, sl])
                    nc.vector.tensor_mul(qs[64:128, :], ps[0:64, :],
                                         ss_sb[64:128, sl])
                    st["ropes"].append([head, sl, qr, qs, None, None])
                for r in st["ropes"]:
                    nc.vector.tensor_add(r[2][:], r[2][:], r[3][:])
                for r in st["ropes"]:
                    q2 = wk_pool.tile([P, 512], bf16, tag="q2", bufs=8)
                    nc.gpsimd.tensor_mul(q2[:], r[2][:], r[2][:])
                    r[4] = q2
                for (head, sl, ps) in items:
                    if head == GH + 1:
                        vt = wk_pool.tile([P, 512], bf16, tag="vt", bufs=4)
                        nc.vector.tensor_copy(vt[:], ps[:])
                        st["vs"].append({"sl": sl, "vt": vt})
                return st

            def tails_pe(st):
                """tail PE ops; injected a few kt into the next wave."""
                for v in st["vs"]:
                    sl, vt = v["sl"], v["vt"]
                    gps4 = pp.tile([P, 4], f32, tag="vtp", bufs=1, name="gps4")
                    for i in range(4):
                        tt = sl.start // P + i
                        nc.tensor.matmul(gps4[:, i:i + 1],
                                         lhsT=xt[0][0:GATE_CH,
                                                    tt * P:(tt + 1) * P],
                                         rhs=wg_sb[:], start=True, stop=True)
                    tpall = pp.tile([P, 512], bf16, tag="vtp", bufs=1,
                                    name="tpall")
                    for i in range(4):
                        nc.tensor.transpose(tpall[:, i * P:(i + 1) * P],
                                            vt[:, i * P:(i + 1) * P],
                                            idr_sb[:])
                    v["gps4"], v["tpall"] = gps4, tpall
                stags = ["ssq"] if st["vs"] else ["ssq", "vtp"]
                for g, r in enumerate(st["ropes"]):
                    # sum of squares over d, broadcast to all partitions
                    ssqb = pp.tile([P, 512], f32, tag=stags[g % len(stags)],
                                   bufs=1, name="ssqb")
                    nc.tensor.matmul(ssqb[:], lhsT=onesb_sb[:], rhs=r[4][:],
                                     start=True, stop=True)
                    r[5] = ssqb

            def tails_rest(st):
                """remaining ACT/DVE tail work; emitted after the next
                wave's matmuls."""
                for v in st["vs"]:
                    gcol4 = wk_pool.tile([P, 4], f32, tag="gcol4", bufs=4)
                    nc.scalar.activation(gcol4[:], v["gps4"][:], AF.Sigmoid)
                    for i in range(4):
                        tt = v["sl"].start // P + i
                        # v = ve2 * sigmoid(g) + v_proj (ve2 pre-scaled by 2)
                        nc.vector.scalar_tensor_tensor(
                            out=vsb[:, tt, :], in0=ve2_sb[:, tt, :],
                            scalar=gcol4[:, i:i + 1],
                            in1=v["tpall"][:, i * P:(i + 1) * P],
                            op0=ALU.mult, op1=ALU.add,
                        )
                for (head, sl, qr, qs, q2, ssqb) in st["ropes"]:
                    srow = w3_pool.tile([P, 512], f32, tag="srow", bufs=2)
                    nc.scalar.activation(srow[:], ssqb[:], AF.Sqrt,
                                         bias=eps_sb[:], scale=1.0 / HD)
                    rrb = w3_pool.tile([P, 512], f32, tag="rrb", bufs=2)
                    nc.vector.reciprocal_approx_fast(rrb[:], srow[:])
                    dest = khat[:, sl] if head == 0 else qhat[:, head - 1, sl]
                    nc.gpsimd.tensor_mul(dest, qr[:], rrb[:])

            # head-major group list: wave 0 is k + v only (8 groups), so the
            # ramp-critical DMA stream per kt is just wk/wv/xt -- the wq
            # slices stream in behind for the later q-head waves.  The two
            # extra wave-0 groups ride the ssq/vtp psum slots, unused until
            # the first tails.
            groups = (
                [(0, t) for t in range(3)] + [(GH + 1, t) for t in range(3)]
                + [(0, 3), (GH + 1, 3)]
                + [(h, t) for h in (1, 2) for t in range(3)]
                + [(h, t) for h in (3, 4) for t in range(3)]
                + [(h, 3) for h in range(1, GH + 1)]
            )
            TAG_BUFS = {"pb": 4, "yp": 2, "ssq": 1, "vtp": 1}
            wave_sizes = [8, 6, 6, 4]
            wave_tags = [["pb"] * 4 + ["yp", "yp", "ssq", "vtp"],
                         ["pb"] * 4 + ["yp", "yp"],
                         ["pb"] * 4 + ["yp", "yp"], ["pb"] * 4]
            pending = None
            w0 = 0
            for wi, wsz in enumerate(wave_sizes):
                st = tails_free(pending) if pending else None
                inject = (lambda s=st: tails_pe(s)) if st else None
                items = wave_mms(groups[w0:w0 + wsz], wave_tags[wi], inject)
                w0 += wsz
                if st:
                    tails_rest(st)
                pending = items
            st = tails_free(pending)
            tails_pe(st)
            tails_rest(st)

            CO = C_ // 512  # output column chunks
            # All 4 q-heads fused into one 512-wide moving operand: scores /
            # exp / PV are each ONE N=512 instruction per (qi, kt).
            yps = {}
            ptsums = {}
            yqs = {}

            def attn_scores_k(qi, kk):
                ktc = min(WT + 1, NT - qi)
                qs4 = qhat[:, :, qi * P:(qi + 1) * P]   # [d, (h, q)] = 512 wide
                kt = qi + kk
                sp = pp.tile([P, GH * P], f32, tag="pb", bufs=4, name="sp")
                nc.tensor.matmul(
                    sp[:], lhsT=khat[:, kt * P:(kt + 1) * P], rhs=qs4,
                    start=True, stop=True,
                )
                pt = aw.tile([P, GH * P], bf16, tag="pT", bufs=5)
                nc.scalar.activation(pt[:], sp[:], AF.Exp, scale=ISQ)
                masked = (kk == 0) or (kk == WT and ktc == WT + 1)
                if masked:
                    ptm = aw.tile([P, GH * P], bf16, tag="ptm", bufs=3)
                    nc.vector.tensor_mul(ptm[:], pt[:],
                                         mlo_sb[:] if kk == 0 else mhi_sb[:])
                    pt = ptm
                # accumulate exp tiles for the softmax denominator as two
                # parallel DVE chains (even/odd kk), always writing a FRESH
                # bf16 tile: in-place accumulation defeats the 16-bit
                # dual-port packing and runs 2x slower
                acc = ptsums.setdefault(qi, {0: None, 1: None})
                par = kk & 1
                eng = nc.vector if par == 0 else nc.gpsimd
                if acc[par] is None:
                    acc[par] = [pt]            # defer: first add pairs two pts
                else:
                    prev = acc[par][0][:] if isinstance(acc[par], list) \
                        else acc[par][:]
                    psum_t = aw.tile([P, GH * P], bf16,
                                     tag=f"ptsum{par}", bufs=3)
                    eng.tensor_add(psum_t[:], prev, pt[:])
                    acc[par] = psum_t
                return pt

            def attn_pv_k(qi, kk, pt):
                ktc = min(WT + 1, NT - qi)
                if kk == 0:
                    yps[qi] = pp.tile([P, GH * P], f32, tag="yp", bufs=2,
                                      name=f"yp{qi}")
                kt = qi + kk
                nc.tensor.matmul(
                    yps[qi][:], lhsT=vsb[:, kt, :], rhs=pt[:],
                    start=(kk == 0), stop=(kk == ktc - 1),
                )
                if kk == ktc - 1:
                    # softmax denominator, part 1: combine the two chains
                    # (DVE); the PE-side all-ones matmul is deferred a couple
                    # of iterations so the PE never waits on this chain
                    a, b = ptsums[qi][0], ptsums[qi][1]
                    if b is None:                          # ktc == 1
                        pts = a[0]
                    else:
                        aa = a[0] if isinstance(a, list) else a
                        bb = b[0] if isinstance(b, list) else b
                        pts = aw.tile([P, GH * P], bf16, tag="ptsum0", bufs=3)
                        nc.vector.tensor_add(pts[:], aa[:], bb[:])
                    den_ready.append([qi, pts, 2])

            def den_emit(qi, pts):
                # all-ones matmul = key-reduction + row broadcast across
                # partitions in one cheap instruction
                denb = pp.tile([P, GH * P], f32, tag="pb", bufs=4,
                               name="denb")
                nc.tensor.matmul(denb[:], lhsT=onesb_sb[:], rhs=pts[:],
                                 start=True, stop=True)
                rdb = w3_pool.tile([P, GH * P], f32, tag="rdb", bufs=2)
                nc.vector.reciprocal_approx_fast(rdb[:], denb[:])
                yq = op_pool.tile([P, GH * P], bf16, tag="yq", bufs=2)
                nc.vector.tensor_mul(yq[:], yps[qi][:], rdb[:])
                yqs[qi] = yq

            def attn_out(qi):
                qsl = slice(qi * P, (qi + 1) * P)
                yq = yqs[qi]
                for co in range(CO):
                    osl = slice(co * 512, co * 512 + 512)
                    ops = pp.tile([P, 512], f32, tag=("ssq", "vtp")[co % 2],
                                  bufs=1, name="ops")
                    for h in range(GH):
                        nc.tensor.matmul(
                            ops[:], lhsT=yq[:, h * P:(h + 1) * P],
                            rhs=wo_sb[:, h, osl],
                            start=(h == 0), stop=(h == GH - 1),
                        )
                    ob = op_pool.tile([P, 512], bf16, tag="ob", bufs=4)
                    if co % 2 == 0:    # split psum drain across ACT and DVE
                        nc.scalar.activation(ob[:], ops[:], AF.Copy)
                    else:
                        nc.vector.tensor_copy(out=ob[:], in_=ops[:])
                    nc.sync.dma_start(out=out_d[qsl, osl], in_=ob[:])

            pv_queue = deque()
            done_out = set()
            out_ready = deque()
            den_ready = deque()

            def tick_dens():
                if den_ready and den_ready[0][2] <= 0:
                    q, pts, _ = den_ready.popleft()
                    den_emit(q, pts)
                for e in den_ready:
                    e[2] -= 1

            def tick_outs():
                # emit out-proj a few iterations after its recip is queued,
                # so the PE never waits on the denominator/yq chain
                if out_ready and out_ready[0][1] <= 0:
                    done_out.add(out_ready[0][0])
                    attn_out(out_ready.popleft()[0])
                for e in out_ready:
                    e[1] -= 1
                for q in list(yqs):
                    if q not in done_out and all(e[0] != q for e in out_ready):
                        out_ready.append([q, 3])

            for qi in range(NT):
                ktc = min(WT + 1, NT - qi)
                for kk in range(ktc):
                    pt = attn_scores_k(qi, kk)
                    if len(pv_queue) >= 3:
                        attn_pv_k(*pv_queue.popleft())
                    pv_queue.append((qi, kk, pt))
                    tick_dens()
                    tick_outs()
            while pv_queue:
                attn_pv_k(*pv_queue.popleft())
                tick_dens()
                tick_outs()
            while den_ready:
                q, pts, _ = den_ready.popleft()
                den_emit(q, pts)
                tick_outs()
            for qi in range(NT):
                if qi not in done_out:
                    attn_out(qi)

    return nc


def _get_program(T_=T, C_=C, win=WINDOW):
    key = (T_, C_, win)
    if key not in _PROGRAM_CACHE:
        nc = build_program(T_, C_, win)
        nc.finalize()
        _PROGRAM_CACHE[key] = nc
    return _PROGRAM_CACHE[key]


def make_in_maps(x, ve, cos, sin, Wq, Wk, Wv, Wg, Wo):
    """Build the 8 per-core input dicts (host-side sharding/layout prep)."""
    cosT = np.ascontiguousarray(cos[:, 0, :].T).astype(np.float32)  # [64, T]
    sinT = np.ascontiguousarray(sin[:, 0, :].T).astype(np.float32)
    cc = np.concatenate([cosT, cosT], axis=0)            # [128, T]
    ss = np.concatenate([sinT, -sinT], axis=0)           # [128, T]
    # 0/1 multiplicative band masks for the S^T diagonal/far tiles,
    # replicated across the 4 fused q heads: [kj, h*128 + q]
    kj = np.arange(P)[:, None]
    qq = np.arange(P)[None, :]
    m_lo = np.tile((kj >= qq).astype(np.float32), (1, GH)).astype(BF16)
    m_hi = np.tile((kj < qq).astype(np.float32), (1, GH)).astype(BF16)
    onesb = np.ones((P, P), dtype=np.float32).astype(BF16)
    identr = np.eye(P, dtype=np.float32).astype(BF16)

    in_maps = []
    for core in range(N_CORES):
        b, g = divmod(core, N_KV)
        in_maps.append({
            "xT": np.ascontiguousarray(x[b].T).astype(BF16),
            "wq": Wq[:, g * GH * HD:(g + 1) * GH * HD].astype(BF16),
            "wk": Wk[:, g * HD:(g + 1) * HD].astype(BF16),
            "wv": Wv[:, g * HD:(g + 1) * HD].astype(BF16),
            "wg": np.ascontiguousarray(Wg[:, g:g + 1]).astype(BF16),
            "ve2": (2.0 * ve[b][:, g * HD:(g + 1) * HD]).astype(BF16),
            "wo": Wo[g * GH * HD:(g + 1) * GH * HD, :].astype(BF16),
            "cc": cc.astype(BF16), "ss": ss.astype(BF16),
            "mlo": m_lo, "mhi": m_hi, "onesb": onesb,
            "identr": identr,
        })
    return in_maps


def kernel(x, ve, cos, sin, Wq, Wk, Wv, Wg, Wo, window):
    assert int(window) == WINDOW and x.shape == (B, T, C)
    from concourse.bass_utils import run_bass_kernel_spmd

    nc = _get_program()
    in_maps = make_in_maps(x, ve, cos, sin, Wq, Wk, Wv, Wg, Wo)
    res = run_bass_kernel_spmd(nc, in_maps, core_ids=list(range(N_CORES)))
    out = np.zeros((B, T, C), dtype=np.float32)
    for core in range(N_CORES):
        b = core // N_KV
        out[b] += np.asarray(res.results[core]["out"], dtype=np.float32)
    return out


# revision 17
# speedup vs baseline: 1.0961x; 1.0961x over previous
"""Trainium2 Bass kernel for a GQA sliding-window attention layer.

Reference computation (B=2, T=2048, C=2048, 16 Q heads / 4 KV heads, d=128):
    q = x @ Wq; k = x @ Wk; v = x @ Wv (+ sigmoid-gated value embedding)
    q, k = rmsnorm(rope(q)), rmsnorm(rope(k))
    scores masked to the band 0 <= j - i < window (=1024), softmax over j
    out = (p @ v) @ Wo

Sharding: 8 cores = 2 batches x 4 KV groups.  Each core computes its 4 Q
heads / 1 KV head for one batch and a partial output (its 512-row slice of
the Wo contraction); the host sums the 4 partials per batch.

Layout strategy per core:
  - xT (C x T, bf16) resident in SBUF; all projections contract over C.
  - q-hat / k-hat kept [d=128 partitions, T free]; scores computed transposed
    (S^T tiles [kj, qi]) so that P^T feeds the PV matmul directly with v in
    natural [token, d] layout.
  - softmax has no max-subtraction: rms-normalized q,k bound |score| by
    sqrt(128); 1/sqrt(d) is folded into the exp's scale operand.
  - softmax denominators: exp tiles are accumulated elementwise (DVE), then
    one all-ones [128,128] f32r matmul per q-tile performs the key-reduction
    AND broadcasts the result across all partitions in one cheap matmul.
    The rms row-sums use the same all-ones trick (no [1,N] psum outputs, no
    separate broadcast matmuls).
  - band masking via 0/1 bf16 multiplies on GpSimd (no PE bias matmuls).
  - projections emitted as kt-major waves (first wave 8 groups wide so the
    PE chases the initial x/weight DMA stream at full rate).  Each wave's
    tail work is split: psum-releasing ops (rope mults) are emitted before
    the next wave's matmuls, tail PE ops are injected mid-wave, and the
    remainder trails the wave, so no engine queue ever deadlocks on a
    recycled psum bank.
"""

import numpy as np
import ml_dtypes
from collections import deque

BF16 = ml_dtypes.bfloat16

# Problem dims (hardcoded per contest rules)
B, T, C = 2, 2048, 2048
N_HEAD, N_KV, HD, GATE_CH = 16, 4, 128, 32
WINDOW = 1024
P = 128
GH = N_HEAD // N_KV  # q heads per kv head (= per core)
N_CORES = 8

_PROGRAM_CACHE = {}


def build_program(T_=T, C_=C, win=WINDOW):
    import concourse.mybir as mybir
    import concourse.tile as tile
    from concourse import bacc

    dt = mybir.dt
    f32 = dt.float32
    bf16 = dt.bfloat16
    AF = mybir.ActivationFunctionType
    ALU = mybir.AluOpType

    NT = T_ // P          # token tiles
    KT = C_ // P          # contraction tiles
    WT = win // P         # window tiles
    TS = T_ // 512        # 512-wide token slices
    ISQ = 1.0 / float(np.sqrt(HD))
    INJ = 8               # kt index where prev wave's tail PE ops interleave

    nc = bacc.Bacc()

    xT = nc.declare_dram_parameter("xT", [C_, T_], bf16, isOutput=False)
    wq = nc.declare_dram_parameter("wq", [C_, GH * HD], bf16, isOutput=False)
    wk = nc.declare_dram_parameter("wk", [C_, HD], bf16, isOutput=False)
    wv = nc.declare_dram_parameter("wv", [C_, HD], bf16, isOutput=False)
    wg = nc.declare_dram_parameter("wg", [GATE_CH, 1], bf16, isOutput=False)
    ve2 = nc.declare_dram_parameter("ve2", [T_, HD], bf16, isOutput=False)
    wo = nc.declare_dram_parameter("wo", [GH * HD, C_], bf16, isOutput=False)
    ccd = nc.declare_dram_parameter("cc", [P, T_], bf16, isOutput=False)
    ssd = nc.declare_dram_parameter("ss", [P, T_], bf16, isOutput=False)
    mlod = nc.declare_dram_parameter("mlo", [P, GH * P], bf16, isOutput=False)
    mhid = nc.declare_dram_parameter("mhi", [P, GH * P], bf16, isOutput=False)
    onebd = nc.declare_dram_parameter("onesb", [P, P], bf16, isOutput=False)
    idr = nc.declare_dram_parameter("identr", [P, P], bf16, isOutput=False)
    out_d = nc.declare_dram_parameter("out", [T_, C_], bf16, isOutput=True)

    with tile.TileContext(nc) as tc:
        with (
            tc.tile_pool(name="singles", bufs=1) as sg,
            tc.tile_pool(name="work", bufs=2) as wk_pool,
            tc.tile_pool(name="work3", bufs=3) as w3_pool,
            tc.tile_pool(name="attw", bufs=4) as aw,
            tc.tile_pool(name="outp", bufs=3) as op_pool,
            tc.tile_pool(name="psum", bufs=6, space="PSUM") as pp,
        ):
            # ---- persistent inputs -------------------------------------
            # small constants FIRST (rope tails read cc/ss early; masks and
            # the all-ones tiles are needed at the start of attention)
            wg_sb = sg.tile([GATE_CH, 1], bf16, tag="wg")
            nc.sync.dma_start(out=wg_sb[:], in_=wg[:])
            cc_sb = sg.tile([P, T_], bf16, tag="cc")
            nc.sync.dma_start(out=cc_sb[:], in_=ccd[:])
            ss_sb = sg.tile([P, T_], bf16, tag="ss")
            nc.sync.dma_start(out=ss_sb[:], in_=ssd[:])
            mlo_sb = sg.tile([P, GH * P], bf16, tag="mlo")
            nc.sync.dma_start(out=mlo_sb[:], in_=mlod[:])
            mhi_sb = sg.tile([P, GH * P], bf16, tag="mhi")
            nc.sync.dma_start(out=mhi_sb[:], in_=mhid[:])
            onesb_sb = sg.tile([P, P], bf16, tag="onesbb")
            nc.sync.dma_start(out=onesb_sb[:], in_=onebd[:])
            idr_sb = sg.tile([P, P], bf16, tag="idr")
            nc.sync.dma_start(out=idr_sb[:], in_=idr[:])
            xt = []
            wq_sb = sg.tile([P, KT, GH * HD], bf16, tag="wq")
            wk_sb = sg.tile([P, KT, HD], bf16, tag="wk")
            wv_sb = sg.tile([P, KT, HD], bf16, tag="wv")
            wqr = wq.rearrange("(o p) n -> p o n", p=P)
            wkr = wk.rearrange("(o p) n -> p o n", p=P)
            wvr = wv.rearrange("(o p) n -> p o n", p=P)
            for kt in range(KT):
                t_ = sg.tile([P, T_], bf16, tag=f"xt{kt}")
                nc.sync.dma_start(out=wk_sb[:, kt, :], in_=wkr[:, kt, :])
                nc.sync.dma_start(out=wv_sb[:, kt, :], in_=wvr[:, kt, :])
                nc.sync.dma_start(out=t_[:], in_=xT[kt * P:(kt + 1) * P, :])
                xt.append(t_)
            for kt in range(KT):
                nc.sync.dma_start(out=wq_sb[:, kt, :], in_=wqr[:, kt, :])
            ve2_sb = sg.tile([P, NT, HD], bf16, tag="ve2")
            nc.sync.dma_start(out=ve2_sb[:], in_=ve2.rearrange("(o p) d -> p o d", p=P))
            wo_sb = sg.tile([P, GH, C_], bf16, tag="wo")
            nc.sync.dma_start(out=wo_sb[:], in_=wo.rearrange("(o p) n -> p o n", p=P))
            eps_sb = sg.tile([P, 1], f32, tag="epsb")
            nc.vector.memset(eps_sb[:], 1e-6)

            # persistent intermediates
            qhat = sg.tile([P, GH, T_], bf16, tag="qhat")   # normalized roped q, [d, h, t]
            khat = sg.tile([P, T_], bf16, tag="khat")       # normalized roped k
            vsb = sg.tile([P, NT, HD], bf16, tag="vsb")     # gated v, [tok, tt, d]

            # ---- projections + rope + rmsnorm for k/q heads and vT -----
            # heads: 0 = k, 1..GH = q heads, GH+1 = v
            def wave_mms(wave, tags, inject):
                items = []
                for (head, ts_), tg in zip(wave, tags):
                    sl = slice(ts_ * 512, ts_ * 512 + 512)
                    ps = pp.tile([P, 512], f32, tag=tg, bufs=TAG_BUFS[tg],
                                 name=f"ps{head}_{ts_}")
                    items.append((head, sl, ps))
                for kt in range(KT):
                    if kt == INJ and inject is not None:
                        inject()
                    for gi, (head, ts_) in enumerate(wave):
                        if head == 0:
                            w_ap = wk_sb[:, kt, :]
                        elif head == GH + 1:
                            w_ap = wv_sb[:, kt, :]
                        else:
                            w_ap = wq_sb[:, kt, (head - 1) * HD:head * HD]
                        nc.tensor.matmul(
                            items[gi][2][:], lhsT=w_ap,
                            rhs=xt[kt][:, items[gi][1]],
                            start=(kt == 0), stop=(kt == KT - 1),
                        )
                return items

            def tails_free(items):
                """psum-releasing rope/copy work; emit BEFORE next wave's
                matmuls so the banks recycle while the next wave runs."""
                st = {"ropes": [], "vs": []}
                for (head, sl, ps) in items:
                    if head == GH + 1:
                        continue
                    # rope: qr = ps*cc + swap(ps)*ss (ss carries the sign);
                    # all on DVE (GpSimd has no PSUM port)
                    qr = w3_pool.tile([P, 512], f32, tag="qr", bufs=8)
                    nc.vector.tensor_mul(qr[:], ps[:], cc_sb[:, sl])
                    qs = wk_pool.tile([P, 512], f32, tag="qs", bufs=2)
                    nc.vector.tensor_mul(qs[0:64, :], ps[64:128, :],
                                         ss_sb[0:64, sl])
                    nc.vector.tensor_mul(qs[64:128, :], ps[0:64, :],
                                         ss_sb[64:128, sl])
                    st["ropes"].append([head, sl, qr, qs, None, None])
                for r in st["ropes"]:
                    nc.vector.tensor_add(r[2][:], r[2][:], r[3][:])
                for r in st["ropes"]:
                    q2 = wk_pool.tile([P, 512], bf16, tag="q2", bufs=8)
                    nc.gpsimd.tensor_mul(q2[:], r[2][:], r[2][:])
                    r[4] = q2
                for (head, sl, ps) in items:
                    if head == GH + 1:
                        vt = wk_pool.tile([P, 512], bf16, tag="vt", bufs=4)
                        nc.vector.tensor_copy(vt[:], ps[:])
                        st["vs"].append({"sl": sl, "vt": vt})
                return st

            def tails_pe(st):
                """tail PE ops; injected a few kt into the next wave."""
                for v in st["vs"]:
                    sl, vt = v["sl"], v["vt"]
                    gps4 = pp.tile([P, 4], f32, tag="vtp", bufs=1, name="gps4")
                    for i in range(4):
                        tt = sl.start // P + i
                        nc.tensor.matmul(gps4[:, i:i + 1],
                                         lhsT=xt[0][0:GATE_CH,
                                                    tt * P:(tt + 1) * P],
                                         rhs=wg_sb[:], start=True, stop=True)
                    tpall = pp.tile([P, 512], bf16, tag="vtp", bufs=1,
                                    name="tpall")
                    for i in range(4):
                        nc.tensor.transpose(tpall[:, i * P:(i + 1) * P],
                                            vt[:, i * P:(i + 1) * P],
                                            idr_sb[:])
                    v["gps4"], v["tpall"] = gps4, tpall
                stags = ["ssq"] if st["vs"] else ["ssq", "vtp"]
                for g, r in enumerate(st["ropes"]):
                    # sum of squares over d, broadcast to all partitions
                    ssqb = pp.tile([P, 512], f32, tag=stags[g % len(stags)],
                                   bufs=1, name="ssqb")
                    nc.tensor.matmul(ssqb[:], lhsT=onesb_sb[:], rhs=r[4][:],
                                     start=True, stop=True)
                    r[5] = ssqb

            def tails_rest(st):
                """remaining ACT/DVE tail work; emitted after the next
                wave's matmuls."""
                for v in st["vs"]:
                    gcol4 = wk_pool.tile([P, 4], f32, tag="gcol4", bufs=4)
                    nc.scalar.activation(gcol4[:], v["gps4"][:], AF.Sigmoid)
                    for i in range(4):
                        tt = v["sl"].start // P + i
                        # v = ve2 * sigmoid(g) + v_proj (ve2 pre-scaled by 2)
                        nc.vector.scalar_tensor_tensor(
                            out=vsb[:, tt, :], in0=ve2_sb[:, tt, :],
                            scalar=gcol4[:, i:i + 1],
                            in1=v["tpall"][:, i * P:(i + 1) * P],
                            op0=ALU.mult, op1=ALU.add,
                        )
                for (head, sl, qr, qs, q2, ssqb) in st["ropes"]:
                    srow = w3_pool.tile([P, 512], f32, tag="srow", bufs=2)
                    nc.scalar.activation(srow[:], ssqb[:], AF.Sqrt,
                                         bias=eps_sb[:], scale=1.0 / HD)
                    rrb = w3_pool.tile([P, 512], f32, tag="rrb", bufs=2)
                    nc.vector.reciprocal_approx_fast(rrb[:], srow[:])
                    dest = khat[:, sl] if head == 0 else qhat[:, head - 1, sl]
                    nc.gpsimd.tensor_mul(dest, qr[:], rrb[:])

            # head-major group list: wave 0 is k + v only (8 groups), so the
            # ramp-critical DMA stream per kt is just wk/wv/xt -- the wq
            # slices stream in behind for the later q-head waves.  The two
            # extra wave-0 groups ride the ssq/vtp psum slots, unused until
            # the first tails.
            groups = (
                [(0, t) for t in range(3)] + [(GH + 1, t) for t in range(3)]
                + [(0, 3), (GH + 1, 3)]
                + [(h, t) for h in (1, 2) for t in range(3)]
                + [(h, t) for h in (3, 4) for t in range(3)]
                + [(h, 3) for h in range(1, GH + 1)]
            )
            TAG_BUFS = {"pb": 4, "yp": 2, "ssq": 1, "vtp": 1}
            wave_sizes = [8, 6, 6, 4]
            wave_tags = [["pb"] * 4 + ["yp", "yp", "ssq", "vtp"],
                         ["pb"] * 4 + ["yp", "yp"],
                         ["pb"] * 4 + ["yp", "yp"], ["pb"] * 4]
            pending = None
            w0 = 0
            for wi, wsz in enumerate(wave_sizes):
                st = tails_free(pending) if pending else None
                inject = (lambda s=st: tails_pe(s)) if st else None
                items = wave_mms(groups[w0:w0 + wsz], wave_tags[wi], inject)
                w0 += wsz
                if st:
                    tails_rest(st)
                pending = items
            st = tails_free(pending)
            tails_pe(st)
            tails_rest(st)

            CO = C_ // 512  # output column chunks
            # All 4 q-heads fused into one 512-wide moving operand: scores /
            # exp / PV are each ONE N=512 instruction per (qi, kt).
            yps = {}
            ptsums = {}
            yqs = {}

            def attn_scores_k(qi, kk):
                ktc = min(WT + 1, NT - qi)
                qs4 = qhat[:, :, qi * P:(qi + 1) * P]   # [d, (h, q)] = 512 wide
                kt = qi + kk
                sp = pp.tile([P, GH * P], f32, tag="pb", bufs=4, name="sp")
                nc.tensor.matmul(
                    sp[:], lhsT=khat[:, kt * P:(kt + 1) * P], rhs=qs4,
                    start=True, stop=True,
                )
                pt = aw.tile([P, GH * P], bf16, tag="pT", bufs=5)
                nc.scalar.activation(pt[:], sp[:], AF.Exp, scale=ISQ)
                masked = (kk == 0) or (kk == WT and ktc == WT + 1)
                if masked:
                    ptm = aw.tile([P, GH * P], bf16, tag="ptm", bufs=3)
                    nc.vector.tensor_mul(ptm[:], pt[:],
                                         mlo_sb[:] if kk == 0 else mhi_sb[:])
                    pt = ptm
                # accumulate exp tiles for the softmax denominator as two
                # parallel DVE chains (even/odd kk), always writing a FRESH
                # bf16 tile: in-place accumulation defeats the 16-bit
                # dual-port packing and runs 2x slower
                acc = ptsums.setdefault(qi, {0: None, 1: None})
                par = kk & 1
                eng = nc.vector if par == 0 else nc.gpsimd
                if acc[par] is None:
                    acc[par] = [pt]            # defer: first add pairs two pts
                else:
                    prev = acc[par][0][:] if isinstance(acc[par], list) \
                        else acc[par][:]
                    psum_t = aw.tile([P, GH * P], bf16,
                                     tag=f"ptsum{par}", bufs=3)
                    eng.tensor_add(psum_t[:], prev, pt[:])
                    acc[par] = psum_t
                return pt

            def attn_pv_k(qi, kk, pt):
                ktc = min(WT + 1, NT - qi)
                if kk == 0:
                    yps[qi] = pp.tile([P, GH * P], f32, tag="yp", bufs=2,
                                      name=f"yp{qi}")
                kt = qi + kk
                nc.tensor.matmul(
                    yps[qi][:], lhsT=vsb[:, kt, :], rhs=pt[:],
                    start=(kk == 0), stop=(kk == ktc - 1),
                )
                if kk == ktc - 1:
                    # softmax denominator, part 1: combine the two chains
                    # (DVE); the PE-side all-ones matmul is deferred a couple
                    # of iterations so the PE never waits on this chain
                    a, b = ptsums[qi][0], ptsums[qi][1]
                    if b is None:                          # ktc == 1
                        pts = a[0]
                    else:
                        aa = a[0] if isinstance(a, list) else a
                        bb = b[0] if isinstance(b, list) else b
                        pts = aw.tile([P, GH * P], bf16, tag="ptsum0", bufs=3)
                        nc.vector.tensor_add(pts[:], aa[:], bb[:])
                    den_ready.append([qi, pts, 2])

            def den_emit(qi, pts):
                # all-ones matmul = key-reduction + row broadcast across
                # partitions in one cheap instruction
                denb = pp.tile([P, GH * P], f32, tag="pb", bufs=4,
                               name="denb")
                nc.tensor.matmul(denb[:], lhsT=onesb_sb[:], rhs=pts[:],
                                 start=True, stop=True)
                rdb = w3_pool.tile([P, GH * P], f32, tag="rdb", bufs=2)
                nc.vector.reciprocal_approx_fast(rdb[:], denb[:])
                yq = op_pool.tile([P, GH * P], bf16, tag="yq", bufs=2)
                nc.vector.tensor_mul(yq[:], yps[qi][:], rdb[:])
                yqs[qi] = yq

            def attn_out(qi):
                qsl = slice(qi * P, (qi + 1) * P)
                yq = yqs[qi]
                for co in range(CO):
                    osl = slice(co * 512, co * 512 + 512)
                    ops = pp.tile([P, 512], f32, tag=("ssq", "vtp")[co % 2],
                                  bufs=1, name="ops")
                    for h in range(GH):
                        nc.tensor.matmul(
                            ops[:], lhsT=yq[:, h * P:(h + 1) * P],
                            rhs=wo_sb[:, h, osl],
                            start=(h == 0), stop=(h == GH - 1),
                        )
                    ob = op_pool.tile([P, 512], bf16, tag="ob", bufs=4)
                    if co % 2 == 0:    # split psum drain across ACT and DVE
                        nc.scalar.activation(ob[:], ops[:], AF.Copy)
                    else:
                        nc.vector.tensor_copy(out=ob[:], in_=ops[:])
                    nc.sync.dma_start(out=out_d[qsl, osl], in_=ob[:])

            pv_queue = deque()
            done_out = set()
            out_ready = deque()
            den_ready = deque()

            def tick_dens():
                if den_ready and den_ready[0][2] <= 0:
                    q, pts, _ = den_ready.popleft()
                    den_emit(q, pts)
                for e in den_ready:
                    e[2] -= 1

            def tick_outs():
                # emit out-proj a few iterations after its recip is queued,
                # so the PE never waits on the denominator/yq chain
                if out_ready and out_ready[0][1] <= 0:
                    done_out.add(out_ready[0][0])
                    attn_out(out_ready.popleft()[0])
                for e in out_ready:
                    e[1] -= 1
                for q in list(yqs):
                    if q not in done_out and all(e[0] != q for e in out_ready):
                        out_ready.append([q, 3])

            for qi in range(NT):
                ktc = min(WT + 1, NT - qi)
                for kk in range(ktc):
                    pt = attn_scores_k(qi, kk)
                    if len(pv_queue) >= 3:
                        attn_pv_k(*pv_queue.popleft())
                    pv_queue.append((qi, kk, pt))
                    tick_dens()
                    tick_outs()
            while pv_queue:
                attn_pv_k(*pv_queue.popleft())
                tick_dens()
                tick_outs()
            while den_ready:
                q, pts, _ = den_ready.popleft()
                den_emit(q, pts)
                tick_outs()
            for qi in range(NT):
                if qi not in done_out:
                    attn_out(qi)

    return nc


def _get_program(T_=T, C_=C, win=WINDOW):
    key = (T_, C_, win)
    if key not in _PROGRAM_CACHE:
        nc = build_program(T_, C_, win)
        nc.finalize()
        _PROGRAM_CACHE[key] = nc
    return _PROGRAM_CACHE[key]


def make_in_maps(x, ve, cos, sin, Wq, Wk, Wv, Wg, Wo):
    """Build the 8 per-core input dicts (host-side sharding/layout prep)."""
    cosT = np.ascontiguousarray(cos[:, 0, :].T).astype(np.float32)  # [64, T]
    sinT = np.ascontiguousarray(sin[:, 0, :].T).astype(np.float32)
    cc = np.concatenate([cosT, cosT], axis=0)            # [128, T]
    ss = np.concatenate([sinT, -sinT], axis=0)           # [128, T]
    # 0/1 multiplicative band masks for the S^T diagonal/far tiles,
    # replicated across the 4 fused q heads: [kj, h*128 + q]
    kj = np.arange(P)[:, None]
    qq = np.arange(P)[None, :]
    m_lo = np.tile((kj >= qq).astype(np.float32), (1, GH)).astype(BF16)
    m_hi = np.tile((kj < qq).astype(np.float32), (1, GH)).astype(BF16)
    onesb = np.ones((P, P), dtype=np.float32).astype(BF16)
    identr = np.eye(P, dtype=np.float32).astype(BF16)

    in_maps = []
    for core in range(N_CORES):
        b, g = divmod(core, N_KV)
        in_maps.append({
            "xT": np.ascontiguousarray(x[b].T).astype(BF16),
            "wq": Wq[:, g * GH * HD:(g + 1) * GH * HD].astype(BF16),
            "wk": Wk[:, g * HD:(g + 1) * HD].astype(BF16),
            "wv": Wv[:, g * HD:(g + 1) * HD].astype(BF16),
            "wg": np.ascontiguousarray(Wg[:, g:g + 1]).astype(BF16),
            "ve2": (2.0 * ve[b][:, g * HD:(g + 1) * HD]).astype(BF16),
            "wo": Wo[g * GH * HD:(g + 1) * GH * HD, :].astype(BF16),
            "cc": cc.astype(BF16), "ss": ss.astype(BF16),
            "mlo": m_lo, "mhi": m_hi, "onesb": onesb,
            "identr": identr,
        })
    return in_maps


def kernel(x, ve, cos, sin, Wq, Wk, Wv, Wg, Wo, window):
    assert int(window) == WINDOW and x.shape == (B, T, C)
    from concourse.bass_utils import run_bass_kernel_spmd

    nc = _get_program()
    in_maps = make_in_maps(x, ve, cos, sin, Wq, Wk, Wv, Wg, Wo)
    res = run_bass_kernel_spmd(nc, in_maps, core_ids=list(range(N_CORES)))
    out = np.zeros((B, T, C), dtype=np.float32)
    for core in range(N_CORES):
        b = core // N_KV
        out[b] += np.asarray(res.results[core]["out"], dtype=np.float32)
    return out


# revision 18
# speedup vs baseline: 1.1036x; 1.0068x over previous
"""Trainium2 Bass kernel for a GQA sliding-window attention layer.

Reference computation (B=2, T=2048, C=2048, 16 Q heads / 4 KV heads, d=128):
    q = x @ Wq; k = x @ Wk; v = x @ Wv (+ sigmoid-gated value embedding)
    q, k = rmsnorm(rope(q)), rmsnorm(rope(k))
    scores masked to the band 0 <= j - i < window (=1024), softmax over j
    out = (p @ v) @ Wo

Sharding: 8 cores = 2 batches x 4 KV groups.  Each core computes its 4 Q
heads / 1 KV head for one batch and a partial output (its 512-row slice of
the Wo contraction); the host sums the 4 partials per batch.

Layout strategy per core:
  - xT (C x T, bf16) resident in SBUF; all projections contract over C.
  - q-hat / k-hat kept [d=128 partitions, T free]; scores computed transposed
    (S^T tiles [kj, qi]) so that P^T feeds the PV matmul directly with v in
    natural [token, d] layout.
  - softmax has no max-subtraction: rms-normalized q,k bound |score| by
    sqrt(128); 1/sqrt(d) is folded into the exp's scale operand.
  - softmax denominators: exp tiles are accumulated elementwise (DVE), then
    one all-ones [128,128] f32r matmul per q-tile performs the key-reduction
    AND broadcasts the result across all partitions in one cheap matmul.
    The rms row-sums use the same all-ones trick (no [1,N] psum outputs, no
    separate broadcast matmuls).
  - band masking via 0/1 bf16 multiplies on GpSimd (no PE bias matmuls).
  - projections emitted as kt-major waves (first wave 8 groups wide so the
    PE chases the initial x/weight DMA stream at full rate).  Each wave's
    tail work is split: psum-releasing ops (rope mults) are emitted before
    the next wave's matmuls, tail PE ops are injected mid-wave, and the
    remainder trails the wave, so no engine queue ever deadlocks on a
    recycled psum bank.
"""

import numpy as np
import ml_dtypes
from collections import deque

BF16 = ml_dtypes.bfloat16

# Problem dims (hardcoded per contest rules)
B, T, C = 2, 2048, 2048
N_HEAD, N_KV, HD, GATE_CH = 16, 4, 128, 32
WINDOW = 1024
P = 128
GH = N_HEAD // N_KV  # q heads per kv head (= per core)
N_CORES = 8

_PROGRAM_CACHE = {}


def build_program(T_=T, C_=C, win=WINDOW):
    import concourse.mybir as mybir
    import concourse.tile as tile
    from concourse import bacc

    dt = mybir.dt
    f32 = dt.float32
    bf16 = dt.bfloat16
    AF = mybir.ActivationFunctionType
    ALU = mybir.AluOpType

    NT = T_ // P          # token tiles
    KT = C_ // P          # contraction tiles
    WT = win // P         # window tiles
    TS = T_ // 512        # 512-wide token slices
    ISQ = 1.0 / float(np.sqrt(HD))
    INJ = 8               # kt index where prev wave's tail PE ops interleave

    nc = bacc.Bacc()

    xT = nc.declare_dram_parameter("xT", [C_, T_], bf16, isOutput=False)
    wq = nc.declare_dram_parameter("wq", [C_, GH * HD], bf16, isOutput=False)
    wk = nc.declare_dram_parameter("wk", [C_, HD], bf16, isOutput=False)
    wv = nc.declare_dram_parameter("wv", [C_, HD], bf16, isOutput=False)
    wg = nc.declare_dram_parameter("wg", [GATE_CH, 1], bf16, isOutput=False)
    ve2 = nc.declare_dram_parameter("ve2", [T_, HD], bf16, isOutput=False)
    wo = nc.declare_dram_parameter("wo", [GH * HD, C_], bf16, isOutput=False)
    ccd = nc.declare_dram_parameter("cc", [P, T_], bf16, isOutput=False)
    ssd = nc.declare_dram_parameter("ss", [P, T_], bf16, isOutput=False)
    mlod = nc.declare_dram_parameter("mlo", [P, GH * P], bf16, isOutput=False)
    mhid = nc.declare_dram_parameter("mhi", [P, GH * P], bf16, isOutput=False)
    onebd = nc.declare_dram_parameter("onesb", [P, P], bf16, isOutput=False)
    idr = nc.declare_dram_parameter("identr", [P, P], bf16, isOutput=False)
    out_d = nc.declare_dram_parameter("out", [T_, C_], bf16, isOutput=True)

    with tile.TileContext(nc) as tc:
        with (
            tc.tile_pool(name="singles", bufs=1) as sg,
            tc.tile_pool(name="work", bufs=2) as wk_pool,
            tc.tile_pool(name="work3", bufs=3) as w3_pool,
            tc.tile_pool(name="attw", bufs=4) as aw,
            tc.tile_pool(name="outp", bufs=3) as op_pool,
            tc.tile_pool(name="psum", bufs=6, space="PSUM") as pp,
        ):
            # ---- persistent inputs -------------------------------------
            # small constants FIRST (rope tails read cc/ss early; masks and
            # the all-ones tiles are needed at the start of attention)
            wg_sb = sg.tile([GATE_CH, 1], bf16, tag="wg")
            nc.sync.dma_start(out=wg_sb[:], in_=wg[:])
            onesb_sb = sg.tile([P, P], bf16, tag="onesbb")
            nc.sync.dma_start(out=onesb_sb[:], in_=onebd[:])
            cc_sb = sg.tile([P, T_], bf16, tag="cc")
            ss_sb = sg.tile([P, T_], bf16, tag="ss")
            mlo_sb = sg.tile([P, GH * P], bf16, tag="mlo")
            mhi_sb = sg.tile([P, GH * P], bf16, tag="mhi")
            idr_sb = sg.tile([P, P], bf16, tag="idr")
            xt = []
            wq_sb = sg.tile([P, KT, GH * HD], bf16, tag="wq")
            wk_sb = sg.tile([P, KT, HD], bf16, tag="wk")
            wv_sb = sg.tile([P, KT, HD], bf16, tag="wv")
            wqr = wq.rearrange("(o p) n -> p o n", p=P)
            wkr = wk.rearrange("(o p) n -> p o n", p=P)
            wvr = wv.rearrange("(o p) n -> p o n", p=P)
            for kt in range(KT):
                t_ = sg.tile([P, T_], bf16, tag=f"xt{kt}")
                nc.sync.dma_start(out=wk_sb[:, kt, :], in_=wkr[:, kt, :])
                nc.sync.dma_start(out=wv_sb[:, kt, :], in_=wvr[:, kt, :])
                nc.sync.dma_start(out=t_[:], in_=xT[kt * P:(kt + 1) * P, :])
                xt.append(t_)
                if kt == 4:
                    # rope/gate constants: needed only once wave-0 tails
                    # start, so they ride behind the first few x tiles
                    nc.sync.dma_start(out=cc_sb[:], in_=ccd[:])
                    nc.sync.dma_start(out=ss_sb[:], in_=ssd[:])
                    nc.sync.dma_start(out=idr_sb[:], in_=idr[:])
            for kt in range(KT):
                nc.sync.dma_start(out=wq_sb[:, kt, :], in_=wqr[:, kt, :])
            nc.sync.dma_start(out=mlo_sb[:], in_=mlod[:])
            nc.sync.dma_start(out=mhi_sb[:], in_=mhid[:])
            ve2_sb = sg.tile([P, NT, HD], bf16, tag="ve2")
            nc.sync.dma_start(out=ve2_sb[:], in_=ve2.rearrange("(o p) d -> p o d", p=P))
            wo_sb = sg.tile([P, GH, C_], bf16, tag="wo")
            nc.sync.dma_start(out=wo_sb[:], in_=wo.rearrange("(o p) n -> p o n", p=P))
            eps_sb = sg.tile([P, 1], f32, tag="epsb")
            nc.vector.memset(eps_sb[:], 1e-6)

            # persistent intermediates
            qhat = sg.tile([P, GH, T_], bf16, tag="qhat")   # normalized roped q, [d, h, t]
            khat = sg.tile([P, T_], bf16, tag="khat")       # normalized roped k
            vsb = sg.tile([P, NT, HD], bf16, tag="vsb")     # gated v, [tok, tt, d]

            # ---- projections + rope + rmsnorm for k/q heads and vT -----
            # heads: 0 = k, 1..GH = q heads, GH+1 = v
            def wave_mms(wave, tags, inject):
                items = []
                for (head, ts_), tg in zip(wave, tags):
                    sl = slice(ts_ * 512, ts_ * 512 + 512)
                    ps = pp.tile([P, 512], f32, tag=tg, bufs=TAG_BUFS[tg],
                                 name=f"ps{head}_{ts_}")
                    items.append((head, sl, ps))
                for kt in range(KT):
                    if kt == INJ and inject is not None:
                        inject()
                    for gi, (head, ts_) in enumerate(wave):
                        if head == 0:
                            w_ap = wk_sb[:, kt, :]
                        elif head == GH + 1:
                            w_ap = wv_sb[:, kt, :]
                        else:
                            w_ap = wq_sb[:, kt, (head - 1) * HD:head * HD]
                        nc.tensor.matmul(
                            items[gi][2][:], lhsT=w_ap,
                            rhs=xt[kt][:, items[gi][1]],
                            start=(kt == 0), stop=(kt == KT - 1),
                        )
                return items

            def tails_free(items):
                """psum-releasing rope/copy work; emit BEFORE next wave's
                matmuls so the banks recycle while the next wave runs."""
                st = {"ropes": [], "vs": []}
                for (head, sl, ps) in items:
                    if head == GH + 1:
                        continue
                    # rope: qr = ps*cc + swap(ps)*ss (ss carries the sign);
                    # all on DVE (GpSimd has no PSUM port)
                    qr = w3_pool.tile([P, 512], f32, tag="qr", bufs=8)
                    nc.vector.tensor_mul(qr[:], ps[:], cc_sb[:, sl])
                    qs = wk_pool.tile([P, 512], f32, tag="qs", bufs=2)
                    nc.vector.tensor_mul(qs[0:64, :], ps[64:128, :],
                                         ss_sb[0:64, sl])
                    nc.vector.tensor_mul(qs[64:128, :], ps[0:64, :],
                                         ss_sb[64:128, sl])
                    st["ropes"].append([head, sl, qr, qs, None, None])
                for r in st["ropes"]:
                    nc.vector.tensor_add(r[2][:], r[2][:], r[3][:])
                for r in st["ropes"]:
                    q2 = wk_pool.tile([P, 512], bf16, tag="q2", bufs=8)
                    nc.gpsimd.tensor_mul(q2[:], r[2][:], r[2][:])
                    r[4] = q2
                for (head, sl, ps) in items:
                    if head == GH + 1:
                        vt = wk_pool.tile([P, 512], bf16, tag="vt", bufs=4)
                        nc.vector.tensor_copy(vt[:], ps[:])
                        st["vs"].append({"sl": sl, "vt": vt})
                return st

            def tails_pe(st):
                """tail PE ops; injected a few kt into the next wave."""
                for v in st["vs"]:
                    sl, vt = v["sl"], v["vt"]
                    gps4 = pp.tile([P, 4], f32, tag="vtp", bufs=1, name="gps4")
                    for i in range(4):
                        tt = sl.start // P + i
                        nc.tensor.matmul(gps4[:, i:i + 1],
                                         lhsT=xt[0][0:GATE_CH,
                                                    tt * P:(tt + 1) * P],
                                         rhs=wg_sb[:], start=True, stop=True)
                    tpall = pp.tile([P, 512], bf16, tag="vtp", bufs=1,
                                    name="tpall")
                    for i in range(4):
                        nc.tensor.transpose(tpall[:, i * P:(i + 1) * P],
                                            vt[:, i * P:(i + 1) * P],
                                            idr_sb[:])
                    v["gps4"], v["tpall"] = gps4, tpall
                stags = ["ssq"] if st["vs"] else ["ssq", "vtp"]
                for g, r in enumerate(st["ropes"]):
                    # sum of squares over d, broadcast to all partitions
                    ssqb = pp.tile([P, 512], f32, tag=stags[g % len(stags)],
                                   bufs=1, name="ssqb")
                    nc.tensor.matmul(ssqb[:], lhsT=onesb_sb[:], rhs=r[4][:],
                                     start=True, stop=True)
                    r[5] = ssqb

            def tails_rest(st):
                """remaining ACT/DVE tail work; emitted after the next
                wave's matmuls."""
                for v in st["vs"]:
                    gcol4 = wk_pool.tile([P, 4], f32, tag="gcol4", bufs=4)
                    nc.scalar.activation(gcol4[:], v["gps4"][:], AF.Sigmoid)
                    for i in range(4):
                        tt = v["sl"].start // P + i
                        # v = ve2 * sigmoid(g) + v_proj (ve2 pre-scaled by 2)
                        nc.vector.scalar_tensor_tensor(
                            out=vsb[:, tt, :], in0=ve2_sb[:, tt, :],
                            scalar=gcol4[:, i:i + 1],
                            in1=v["tpall"][:, i * P:(i + 1) * P],
                            op0=ALU.mult, op1=ALU.add,
                        )
                for (head, sl, qr, qs, q2, ssqb) in st["ropes"]:
                    srow = w3_pool.tile([P, 512], f32, tag="srow", bufs=2)
                    nc.scalar.activation(srow[:], ssqb[:], AF.Sqrt,
                                         bias=eps_sb[:], scale=1.0 / HD)
                    rrb = w3_pool.tile([P, 512], f32, tag="rrb", bufs=2)
                    nc.vector.reciprocal_approx_fast(rrb[:], srow[:])
                    dest = khat[:, sl] if head == 0 else qhat[:, head - 1, sl]
                    nc.gpsimd.tensor_mul(dest, qr[:], rrb[:])

            # head-major group list: wave 0 is k + v only (8 groups), so the
            # ramp-critical DMA stream per kt is just wk/wv/xt -- the wq
            # slices stream in behind for the later q-head waves.  The two
            # extra wave-0 groups ride the ssq/vtp psum slots, unused until
            # the first tails.
            groups = (
                [(0, t) for t in range(3)] + [(GH + 1, t) for t in range(3)]
                + [(0, 3), (GH + 1, 3)]
                + [(h, t) for h in (1, 2) for t in range(3)]
                + [(h, t) for h in (3, 4) for t in range(3)]
                + [(h, 3) for h in range(1, GH + 1)]
            )
            TAG_BUFS = {"pb": 4, "yp": 2, "ssq": 1, "vtp": 1}
            wave_sizes = [8, 6, 6, 4]
            wave_tags = [["pb"] * 4 + ["yp", "yp", "ssq", "vtp"],
                         ["pb"] * 4 + ["yp", "yp"],
                         ["pb"] * 4 + ["yp", "yp"], ["pb"] * 4]
            pending = None
            w0 = 0
            for wi, wsz in enumerate(wave_sizes):
                st = tails_free(pending) if pending else None
                inject = (lambda s=st: tails_pe(s)) if st else None
                items = wave_mms(groups[w0:w0 + wsz], wave_tags[wi], inject)
                w0 += wsz
                if st:
                    tails_rest(st)
                pending = items
            st = tails_free(pending)
            tails_pe(st)
            tails_rest(st)

            CO = C_ // 512  # output column chunks
            DENSE_Q = 4     # qi <= DENSE_Q: denominator via PE accumulation
                            # (the early-attention window is DVE-bound while
                            # the PE still has slack)
            # All 4 q-heads fused into one 512-wide moving operand: scores /
            # exp / PV are each ONE N=512 instruction per (qi, kt).
            yps = {}
            ptsums = {}
            yqs = {}

            def attn_scores_k(qi, kk):
                ktc = min(WT + 1, NT - qi)
                qs4 = qhat[:, :, qi * P:(qi + 1) * P]   # [d, (h, q)] = 512 wide
                kt = qi + kk
                sp = pp.tile([P, GH * P], f32, tag="pb", bufs=4, name="sp")
                nc.tensor.matmul(
                    sp[:], lhsT=khat[:, kt * P:(kt + 1) * P], rhs=qs4,
                    start=True, stop=True,
                )
                pt = aw.tile([P, GH * P], bf16, tag="pT", bufs=5)
                nc.scalar.activation(pt[:], sp[:], AF.Exp, scale=ISQ)
                masked = (kk == 0) or (kk == WT and ktc == WT + 1)
                if masked:
                    ptm = aw.tile([P, GH * P], bf16, tag="ptm", bufs=3)
                    nc.vector.tensor_mul(ptm[:], pt[:],
                                         mlo_sb[:] if kk == 0 else mhi_sb[:])
                    pt = ptm
                if qi <= DENSE_Q:
                    return pt       # denominator accumulated on the PE
                # accumulate exp tiles for the softmax denominator as two
                # parallel chains (even kk on DVE, odd on GpSimd), always
                # writing a FRESH bf16 tile: in-place accumulation defeats
                # the 16-bit dual-port packing and runs 2x slower
                acc = ptsums.setdefault(qi, {0: None, 1: None})
                par = kk & 1
                eng = nc.vector if par == 0 else nc.gpsimd
                if acc[par] is None:
                    acc[par] = [pt]            # defer: first add pairs two pts
                else:
                    prev = acc[par][0][:] if isinstance(acc[par], list) \
                        else acc[par][:]
                    psum_t = aw.tile([P, GH * P], bf16,
                                     tag=f"ptsum{par}", bufs=3)
                    eng.tensor_add(psum_t[:], prev, pt[:])
                    acc[par] = psum_t
                return pt

            den_acc = {}

            def attn_pv_k(qi, kk, pt):
                ktc = min(WT + 1, NT - qi)
                if kk == 0:
                    yps[qi] = pp.tile([P, GH * P], f32, tag="yp", bufs=2,
                                      name=f"yp{qi}")
                    if qi <= DENSE_Q:
                        den_acc[qi] = pp.tile([P, GH * P], f32, tag="yp",
                                              bufs=2, name=f"dn{qi}")
                kt = qi + kk
                nc.tensor.matmul(
                    yps[qi][:], lhsT=vsb[:, kt, :], rhs=pt[:],
                    start=(kk == 0), stop=(kk == ktc - 1),
                )
                if qi <= DENSE_Q:
                    nc.tensor.matmul(
                        den_acc[qi][:], lhsT=onesb_sb[:], rhs=pt[:],
                        start=(kk == 0), stop=(kk == ktc - 1),
                    )
                    if kk == ktc - 1:
                        rdb = w3_pool.tile([P, GH * P], f32, tag="rdb",
                                           bufs=2)
                        nc.vector.reciprocal_approx_fast(rdb[:],
                                                         den_acc[qi][:])
                        yq = op_pool.tile([P, GH * P], bf16, tag="yq",
                                          bufs=2)
                        nc.vector.tensor_mul(yq[:], yps[qi][:], rdb[:])
                        yqs[qi] = yq
                    return
                if kk == ktc - 1:
                    # softmax denominator, part 1: combine the two chains
                    # (DVE); the PE-side all-ones matmul is deferred a couple
                    # of iterations so the PE never waits on this chain
                    a, b = ptsums[qi][0], ptsums[qi][1]
                    if b is None:                          # ktc == 1
                        pts = a[0]
                    else:
                        aa = a[0] if isinstance(a, list) else a
                        bb = b[0] if isinstance(b, list) else b
                        pts = aw.tile([P, GH * P], bf16, tag="ptsum0", bufs=3)
                        nc.vector.tensor_add(pts[:], aa[:], bb[:])
                    den_ready.append([qi, pts, 2])

            def den_emit(qi, pts):
                # all-ones matmul = key-reduction + row broadcast across
                # partitions in one cheap instruction
                denb = pp.tile([P, GH * P], f32, tag="pb", bufs=4,
                               name="denb")
                nc.tensor.matmul(denb[:], lhsT=onesb_sb[:], rhs=pts[:],
                                 start=True, stop=True)
                rdb = w3_pool.tile([P, GH * P], f32, tag="rdb", bufs=2)
                nc.vector.reciprocal_approx_fast(rdb[:], denb[:])
                yq = op_pool.tile([P, GH * P], bf16, tag="yq", bufs=2)
                nc.vector.tensor_mul(yq[:], yps[qi][:], rdb[:])
                yqs[qi] = yq

            def attn_out(qi):
                qsl = slice(qi * P, (qi + 1) * P)
                yq = yqs[qi]
                for co in range(CO):
                    osl = slice(co * 512, co * 512 + 512)
                    ops = pp.tile([P, 512], f32, tag=("ssq", "vtp")[co % 2],
                                  bufs=1, name="ops")
                    for h in range(GH):
                        nc.tensor.matmul(
                            ops[:], lhsT=yq[:, h * P:(h + 1) * P],
                            rhs=wo_sb[:, h, osl],
                            start=(h == 0), stop=(h == GH - 1),
                        )
                    ob = op_pool.tile([P, 512], bf16, tag="ob", bufs=4)
                    if co % 2 == 0:    # split psum drain across ACT and DVE
                        nc.scalar.activation(ob[:], ops[:], AF.Copy)
                    else:
                        nc.vector.tensor_copy(out=ob[:], in_=ops[:])
                    nc.sync.dma_start(out=out_d[qsl, osl], in_=ob[:])

            pv_queue = deque()
            done_out = set()
            out_ready = deque()
            den_ready = deque()

            def tick_dens():
                if den_ready and den_ready[0][2] <= 0:
                    q, pts, _ = den_ready.popleft()
                    den_emit(q, pts)
                for e in den_ready:
                    e[2] -= 1

            def tick_outs():
                # emit out-proj a few iterations after its recip is queued,
                # so the PE never waits on the denominator/yq chain
                if out_ready and out_ready[0][1] <= 0:
                    done_out.add(out_ready[0][0])
                    attn_out(out_ready.popleft()[0])
                for e in out_ready:
                    e[1] -= 1
                for q in list(yqs):
                    if q not in done_out and all(e[0] != q for e in out_ready):
                        out_ready.append([q, 3])

            for qi in range(NT):
                ktc = min(WT + 1, NT - qi)
                for kk in range(ktc):
                    pt = attn_scores_k(qi, kk)
                    if len(pv_queue) >= 3:
                        attn_pv_k(*pv_queue.popleft())
                    pv_queue.append((qi, kk, pt))
                    tick_dens()
                    tick_outs()
            while pv_queue:
                attn_pv_k(*pv_queue.popleft())
                tick_dens()
                tick_outs()
            while den_ready:
                q, pts, _ = den_ready.popleft()
                den_emit(q, pts)
                tick_outs()
            for qi in range(NT):
                if qi not in done_out:
                    attn_out(qi)

    return nc


def _get_program(T_=T, C_=C, win=WINDOW):
    key = (T_, C_, win)
    if key not in _PROGRAM_CACHE:
        nc = build_program(T_, C_, win)
        nc.finalize()
        _PROGRAM_CACHE[key] = nc
    return _PROGRAM_CACHE[key]


def make_in_maps(x, ve, cos, sin, Wq, Wk, Wv, Wg, Wo):
    """Build the 8 per-core input dicts (host-side sharding/layout prep)."""
    cosT = np.ascontiguousarray(cos[:, 0, :].T).astype(np.float32)  # [64, T]
    sinT = np.ascontiguousarray(sin[:, 0, :].T).astype(np.float32)
    cc = np.concatenate([cosT, cosT], axis=0)            # [128, T]
    ss = np.concatenate([sinT, -sinT], axis=0)           # [128, T]
    # 0/1 multiplicative band masks for the S^T diagonal/far tiles,
    # replicated across the 4 fused q heads: [kj, h*128 + q]
    kj = np.arange(P)[:, None]
    qq = np.arange(P)[None, :]
    m_lo = np.tile((kj >= qq).astype(np.float32), (1, GH)).astype(BF16)
    m_hi = np.tile((kj < qq).astype(np.float32), (1, GH)).astype(BF16)
    onesb = np.ones((P, P), dtype=np.float32).astype(BF16)
    identr = np.eye(P, dtype=np.float32).astype(BF16)

    in_maps = []
    for core in range(N_CORES):
        b, g = divmod(core, N_KV)
        in_maps.append({
            "xT": np.ascontiguousarray(x[b].T).astype(BF16),
            "wq": Wq[:, g * GH * HD:(g + 1) * GH * HD].astype(BF16),
            "wk": Wk[:, g * HD:(g + 1) * HD].astype(BF16),
            "wv": Wv[:, g * HD:(g + 1) * HD].astype(BF16),
            "wg": np.ascontiguousarray(Wg[:, g:g + 1]).astype(BF16),
            "ve2": (2.0 * ve[b][:, g * HD:(g + 1) * HD]).astype(BF16),
            "wo": Wo[g * GH * HD:(g + 1) * GH * HD, :].astype(BF16),
            "cc": cc.astype(BF16), "ss": ss.astype(BF16),
            "mlo": m_lo, "mhi": m_hi, "onesb": onesb,
            "identr": identr,
        })
    return in_maps


def kernel(x, ve, cos, sin, Wq, Wk, Wv, Wg, Wo, window):
    assert int(window) == WINDOW and x.shape == (B, T, C)
    from concourse.bass_utils import run_bass_kernel_spmd

    nc = _get_program()
    in_maps = make_in_maps(x, ve, cos, sin, Wq, Wk, Wv, Wg, Wo)
    res = run_bass_kernel_spmd(nc, in_maps, core_ids=list(range(N_CORES)))
    out = np.zeros((B, T, C), dtype=np.float32)
    for core in range(N_CORES):
        b = core // N_KV
        out[b] += np.asarray(res.results[core]["out"], dtype=np.float32)
    return out
